# revision 31
# baseline (speedup 1.0000x reference)
"""Trainium2 Bass kernel for nn_CrossAttention (linear cross-attention block).

Computation (per batch b):
  xn  = LN(x[b]; norm_g, norm_b)                 [T, D]
  xfn = LN(xf[b]; tnorm_g, tnorm_b)              [N, TD]
  q   = softmax_c((xn @ Wq + bq).reshape(T,H,C))
  k   = softmax_n((xfn @ Wk + bk).reshape(N,H,C))
  v   = (xfn @ Wv + bv).reshape(N,H,C)
  attn= einsum('nhc,nhd->hcd', k, v); y = einsum('thc,hcd->thd', q, attn)
  e   = silu(emb) @ emb_W + emb_b; scale, shift = split(e)
  h   = LN(y; fnorm_g, fnorm_b) * (1+scale) + shift
  out = x + silu(h) @ out_W + out_b

Sharding: pure data-parallel over batch B=32 across 8 NeuronCores (4 each).

Device strategy:
  - LN gain folded into projection weights on the host. x / xf are
    normalized on-chip in natural layout (per-partition -mu / 1/std via one
    DVE tensor_scalar per tile), so projections are plain matmuls and the
    exp/copy PSUM drains need no per-row scale. Bias rank-1 folds are only
    emitted when biases are nonzero (they are zero for this model).
  - The scalar engine stays on ONE activation table set (exp_and_others:
    exp/tanh/identity/copy/square) for the whole kernel: 1/sqrt(var+eps)
    is computed on DVE with the bit-trick + 2 Newton steps, and silu(x) is
    x/2*(1+tanh(x/2)) with the 1/2 folded into the FiLM A/B coefficients
    (host passes fnorm_g/2, fnorm_b/2). This removes ~110 ACT_TABLE_LOADs
    (~1.5us each) that otherwise thrash between exp/sqrt/silu sets.
  - All transposes via the DMA xbar engine with multi-block destinations:
    one instruction per [128, D] tile ([128, KD, 128] dst), not one per
    128x128 block: flat ~1.2us sync-queue cost per instruction.
  - t-tile loop is software-pipelined with a 2-deep skew: per iteration the
    PE runs Q-proj(ti), y-matmul(ti-1), out-proj(ti-2) so the exp/softmax/
    transpose and LN/FiLM/silu/transpose tails of each tile hide under the
    next tiles' matmuls and the PE stays warm (HAM K=8/8). The next batch's
    x/xf loads + LN stats are spread across the current batch's t-loop.
  - Dual softmax: q-softmax over C is a grouped free-dim reduce + broadcast
    multiply; k-softmax over N folds into a per-partition reciprocal scale
    of the (exp_k^T v) head matmuls.
  - Residual uses the raw bf16 x kept resident in SBUF (ACT drains the out
    PSUM, DVE adds at bf16 2x rate); output stored bf16, upcast on host.
"""

from contextlib import ExitStack

import numpy as np
import ml_dtypes

import concourse.bass as bass
import concourse.mybir as mybir
import concourse.tile as tile
from concourse import bacc, library_config
from concourse.bass_utils import run_bass_kernel_spmd
from concourse.masks import make_identity

# problem shapes (hardcoded per contract)
B, T, N, D, TD, H, C, TE = 32, 1024, 256, 1024, 768, 16, 64, 2048
D2 = 2 * D
EPS = 1e-5
NCORES = 8
BPC = B // NCORES           # batches per core
TI = T // 128               # 8 t-tiles
KD = D // 128               # 8 k-tiles over D
KTD = TD // 128             # 6 k-tiles over TD
KTE = TE // 128             # 16 k-tiles over TE
NT = N // 128               # 2 n-tiles
NCH = D // 512              # 2 free 512-chunks over D

F32 = mybir.dt.float32
BF16 = mybir.dt.bfloat16
I32 = mybir.dt.int32
F8 = mybir.dt.float8e4
AF = mybir.ActivationFunctionType
ALU = mybir.AluOpType
NBF = ml_dtypes.bfloat16
NF8 = ml_dtypes.float8_e4m3
MAGIC = 0x5F3759DF

_PROGRAMS = {}  # cached (nc) builds keyed by bias flags


def _build_program(qkv_bias, out_bias, emb_bias):
    nc = bacc.Bacc("TRN2", target_bir_lowering=False, debug=False,
                   num_devices=NCORES)

    # ---- DRAM I/O ----
    d_xbf = nc.dram_tensor("xbf", [BPC, T, D], BF16, kind="ExternalInput")
    d_xfbf = nc.dram_tensor("xfbf", [BPC, N, TD], BF16, kind="ExternalInput")
    d_emb = nc.dram_tensor("emb", [BPC, TE], BF16, kind="ExternalInput")
    d_wq = nc.dram_tensor("wq", [D, D], F8, kind="ExternalInput")
    d_wk = nc.dram_tensor("wk", [TD, D], BF16, kind="ExternalInput")
    d_wv = nc.dram_tensor("wv", [TD, D], BF16, kind="ExternalInput")
    d_wo = nc.dram_tensor("wo", [D, D], BF16, kind="ExternalInput")
    d_wemb = nc.dram_tensor("wemb", [TE, D2], BF16, kind="ExternalInput")
    d_bqe = nc.dram_tensor("bqe", [D], BF16, kind="ExternalInput")
    d_bke = nc.dram_tensor("bke", [D], BF16, kind="ExternalInput")
    d_bve = nc.dram_tensor("bve", [D], BF16, kind="ExternalInput")
    d_outb = nc.dram_tensor("outb", [D], BF16, kind="ExternalInput")
    d_embb = nc.dram_tensor("embb", [D2], BF16, kind="ExternalInput")
    d_fg = nc.dram_tensor("fg", [D], BF16, kind="ExternalInput")   # fnorm_g/2
    d_fb = nc.dram_tensor("fb", [D], BF16, kind="ExternalInput")   # fnorm_b/2
    d_out = nc.dram_tensor("out", [BPC, T, D], BF16, kind="ExternalOutput")

    with tile.TileContext(nc) as tc, ExitStack() as ctx:
        wpool = ctx.enter_context(tc.tile_pool(name="weights", bufs=1))
        cpool = ctx.enter_context(tc.tile_pool(name="consts", bufs=1))

        # ---- persistent weights (K/V first so batch-0 KV starts early) ----
        wk_sb = wpool.tile([128, KTD, D], BF16)
        nc.sync.dma_start(wk_sb[:], d_wk[:].rearrange("(i p) m -> p i m", p=128))
        wv_sb = wpool.tile([128, KTD, D], BF16)
        nc.sync.dma_start(wv_sb[:], d_wv[:].rearrange("(i p) m -> p i m", p=128))

        # ---- constants ----
        def row_const(dram, n, tag):
            t = cpool.tile([1, n], BF16, tag=tag)
            nc.sync.dma_start(t[:], dram[None, :])
            return t
        bqe_r = row_const(d_bqe, D, "bqe_r") if qkv_bias else None
        bke_r = row_const(d_bke, D, "bke_r") if qkv_bias else None
        bve_r = row_const(d_bve, D, "bve_r") if qkv_bias else None
        outb_r = row_const(d_outb, D, "outb_r") if out_bias else None
        ones_r = cpool.tile([1, 128], BF16)
        nc.vector.memset(ones_r[:], 1.0)
        ones_c = cpool.tile([128, 1], BF16)
        nc.vector.memset(ones_c[:], 1.0)
        ones2 = cpool.tile([128, 2], BF16)
        nc.vector.memset(ones2[:], 0.0)
        nc.vector.memset(ones2[0:64, 0:1], 1.0)
        nc.vector.memset(ones2[64:128, 1:2], 1.0)
        magic_c = cpool.tile([128, TI], I32)
        nc.vector.memset(magic_c[:], MAGIC)
        ident = cpool.tile([128, 128], BF16)
        make_identity(nc, ident[:])
        nc.gpsimd.load_library(library_config.attnmlp)

        a_bf = cpool.tile([BPC, D], BF16)   # FiLM A/2 rows (one per batch)
        b_bf = cpool.tile([BPC, D], BF16)   # FiLM B/2 rows

        # ---- batch-phase pools (prep + KV only; t-loop pools open after
        # the emb phase releases its scratch) ----
        xp = ctx.enter_context(tc.tile_pool(name="x", bufs=2))
        xfp = ctx.enter_context(tc.tile_pool(name="xf", bufs=2))
        statp = ctx.enter_context(tc.tile_pool(name="stat", bufs=2))
        kvp = ctx.enter_context(tc.tile_pool(name="kv", bufs=2))
        abp = ctx.enter_context(tc.tile_pool(name="ab", bufs=2))
        colp = ctx.enter_context(tc.tile_pool(name="cols", bufs=2))

        def emit_rsqrt(out, var_src, w, tag, iters=2, eng=None):
            """out[128, w] f32 = 1/sqrt(var_src + EPS).

            Bit-trick + Newton steps so the ACT engine never has to swap
            to the sqrt table set. 1 iter: ~0.2% rel err."""
            if eng is None:
                eng = nc.vector
            ve = colp.tile([128, w], F32, tag=tag + "ve", name=tag + "ve")
            eng.tensor_scalar_add(ve[:], var_src, EPS)
            t1 = colp.tile([128, w], I32, tag=tag + "t1", name=tag + "t1")
            eng.tensor_scalar(t1[:], ve[:].bitcast(I32), 1, None,
                              op0=ALU.arith_shift_right)
            eng.scalar_tensor_tensor(out.bitcast(I32), magic_c[:, :w],
                                     0, t1[:], op0=ALU.add,
                                     op1=ALU.subtract)
            aa = colp.tile([128, w], F32, tag=tag + "aa", name=tag + "aa")
            cc = colp.tile([128, w], F32, tag=tag + "cc", name=tag + "cc")
            for _ in range(iters):
                eng.tensor_mul(aa[:], ve[:], out)
                eng.tensor_mul(aa[:], aa[:], out)
                eng.tensor_scalar(cc[:], aa[:], -0.5, 1.5,
                                  op0=ALU.mult, op1=ALU.add)
                eng.tensor_mul(out, out, cc[:])

        # ---------- per-batch prep, split into chunks for pipelining ------
        def prep_ab(st, b):
            # must be emitted after the emb phase has written a_bf/b_bf --
            # Tile dependencies only flow forward in trace order
            arow = abp.tile([1, D], BF16, tag="arow", name="arow")
            nc.sync.dma_start(arow[:], a_bf[b:b + 1, :])
            brow = abp.tile([1, D], BF16, tag="brow", name="brow")
            nc.sync.dma_start(brow[:], b_bf[b:b + 1, :])
            st["a_bc"] = abp.tile([128, D], BF16, tag="abc", name="abc")
            nc.gpsimd.partition_broadcast(st["a_bc"][:], arow[:], channels=128)
            st["b_bc"] = abp.tile([128, D], BF16, tag="bbc", name="bbc")
            nc.gpsimd.partition_broadcast(st["b_bc"][:], brow[:], channels=128)

        def prep_loads(b):
            st = {}
            st["xf_nat"] = xfp.tile([128, NT, TD], BF16, tag="xfnat",
                                    name="xfnat")
            nc.sync.dma_start(st["xf_nat"][:],
                              d_xfbf[b].rearrange("(i p) m -> p i m", p=128))
            st["x_nat"] = xp.tile([128, TI, D], BF16, tag="xnat", name="xnat")
            nc.sync.dma_start(st["x_nat"][:],
                              d_xbf[b].rearrange("(i p) m -> p i m", p=128))
            return st

        def prep_xf_stats(st):
            xf_nat = st["xf_nat"]
            mvf = statp.tile([128, NT, 2], F32, tag="mvf", name="mvf")
            st["mvf"] = mvf
            for i in range(NT):
                bst = statp.tile([128, 3, 6], F32, tag="bnstf", name="bnstf")
                for g in range(3):
                    nc.vector.bn_stats(bst[:, g, :],
                                       xf_nat[:, i, g * 256:(g + 1) * 256])
                nc.vector.bn_aggr(mvf[:, i, :], bst[:])

        def prep_xf(st):
            xf_nat = st["xf_nat"]
            mvf = st["mvf"]
            rstdf = statp.tile([128, NT], F32, tag="rstdf", name="rstdf")
            emit_rsqrt(rstdf[:], mvf[:, :, 1], NT, "f")
            bxf = statp.tile([128, NT], F32, tag="bxf", name="bxf")
            nc.vector.scalar_tensor_tensor(bxf[:], mvf[:, :, 0], -1.0,
                                           rstdf[:], op0=ALU.mult,
                                           op1=ALU.mult)
            st["xfnT"] = kvp.tile([128, NT, KTD, 128], BF16, tag="xfnT",
                                  name="xfnT")
            for i in range(NT):
                xfn_t = xfp.tile([128, TD], BF16, tag="xfn", name="xfn")
                nc.scalar.activation(xfn_t[:], xf_nat[:, i, :], AF.Identity,
                                     bias=bxf[:, i:i + 1],
                                     scale=rstdf[:, i:i + 1])
                nc.sync.dma_start_transpose(st["xfnT"][:, i], xfn_t[:])

        def prep_xstats(st, i0, i1):
            if "mvx" not in st:
                st["mvx"] = statp.tile([128, TI, 2], F32, tag="mvx",
                                       name="mvx")
            for i in range(i0, i1):
                bst = statp.tile([128, 2, 6], F32, tag="bnstx", name="bnstx")
                for g in range(2):
                    nc.vector.bn_stats(bst[:, g, :],
                                       st["x_nat"][:, i, g * 512:(g + 1) * 512])
                nc.vector.bn_aggr(st["mvx"][:, i, :], bst[:])

        def prep_xfinish(st):
            st["rstdx"] = statp.tile([128, TI], F32, tag="rstdx", name="rstdx")
            emit_rsqrt(st["rstdx"][:], st["mvx"][:, :, 1], TI, "x")
            st["bx"] = statp.tile([128, TI], F32, tag="bx", name="bx")
            nc.vector.scalar_tensor_tensor(st["bx"][:], st["mvx"][:, :, 0],
                                           -1.0, st["rstdx"][:],
                                           op0=ALU.mult, op1=ALU.mult)

        def emit_xnT(st, i):
            xn_t = xnp.tile([128, D], BF16, tag="xn", name="xn")
            nc.scalar.activation(xn_t[:], st["x_nat"][:, i, :], AF.Identity,
                                 bias=st["bx"][:, i:i + 1],
                                 scale=st["rstdx"][:, i:i + 1])
            t = xntp.tile([128, KD, 128], BF16, tag="xnT", name="xnT")
            nc.sync.dma_start_transpose(t[:], xn_t[:])
            # fp8 copy for the DoubleRow Q-projection (xbar can't move fp8)
            t8 = xntp.tile([128, KD, 128], F8, tag="xnT8", name="xnT8")
            nc.scalar.copy(t8[:], t[:])
            return t8

        def emit_kv_proj(st, nt):
            xfnT = st["xfnT"]
            if nt == 0:
                st["exp_k"] = kvp.tile([128, NT, D], BF16, tag="expk",
                                       name="expk")
                st["v_sb"] = kvp.tile([128, NT, D], BF16, tag="vsb",
                                      name="vsb")
            exp_k, v_sb = st["exp_k"], st["v_sb"]
            if True:
                for ch in range(NCH):
                    cs = slice(ch * 512, (ch + 1) * 512)
                    pk = psq.tile([128, 512], F32, tag="ps", name="pk")
                    for kt in range(KTD):
                        nc.tensor.matmul(pk[:], xfnT[:, nt, kt, :],
                                         wk_sb[:, kt, cs],
                                         start=(kt == 0),
                                         stop=(kt == KTD - 1 and not qkv_bias))
                    if qkv_bias:
                        nc.tensor.matmul(pk[:], ones_r[:], bke_r[0:1, cs],
                                         start=False, stop=True)
                    nc.scalar.activation(exp_k[:, nt, cs], pk[:], AF.Exp)
                    pv = psq.tile([128, 512], F32, tag="ps", name="pv")
                    for kt in range(KTD):
                        nc.tensor.matmul(pv[:], xfnT[:, nt, kt, :],
                                         wv_sb[:, kt, cs],
                                         start=(kt == 0),
                                         stop=(kt == KTD - 1 and not qkv_bias))
                    if qkv_bias:
                        nc.tensor.matmul(pv[:], ones_r[:], bve_r[0:1, cs],
                                         start=False, stop=True)
                    nc.scalar.copy(v_sb[:, nt, cs], pv[:])

        def emit_kv_attn(st):
            exp_k, v_sb = st["exp_k"], st["v_sb"]
            pks = psa.tile([128, KD], F32, tag="kvps", name="pks")
            for j in range(KD):
                for nt in range(NT):
                    nc.tensor.matmul(pks[:, j:j + 1],
                                     exp_k[:, nt, j * 128:(j + 1) * 128],
                                     ones_c[:], start=(nt == 0), stop=(nt == 1))
            r_k = statp.tile([128, KD], F32, tag="rk", name="rk")
            nc.vector.reciprocal(r_k[:], pks[:])

            patt = psa.tile([128, 512], F32, tag="kvps", name="patt")
            for h in range(H):
                rp = slice((h % 2) * 64, (h % 2) * 64 + 64)
                cp = slice((h // 2) * 64, (h // 2) * 64 + 64)
                hs = slice(h * 64, (h + 1) * 64)
                for nt in range(NT):
                    nc.tensor.matmul(patt[rp, cp], exp_k[:, nt, hs],
                                     v_sb[:, nt, hs],
                                     start=(nt == 0), stop=(nt == 1))
            # block-diagonal per head pair: [0:64,0:64]=head 2j, [64:,64:]=2j+1
            attn_s = kvp.tile([128, KD, 128], BF16, tag="attns", name="attns")
            nc.vector.memset(attn_s[:], 0.0)
            for j in range(KD):
                nc.vector.tensor_scalar_mul(attn_s[0:64, j, 0:64],
                                            patt[0:64, j * 64:(j + 1) * 64],
                                            r_k[0:64, j:j + 1])
                nc.vector.tensor_scalar_mul(attn_s[64:128, j, 64:128],
                                            patt[64:128, j * 64:(j + 1) * 64],
                                            r_k[64:128, j:j + 1])
            st["attn_s"] = attn_s

        # ---- prefetch batch 0 (before the emb phase holds up the queue) ----
        psq = ctx.enter_context(
            tc.tile_pool(name="psq", bufs=2, space=bass.MemorySpace.PSUM))
        psa = ctx.enter_context(
            tc.tile_pool(name="psa", bufs=1, space=bass.MemorySpace.PSUM))

        prep = prep_loads(0)
        prep_xf_stats(prep)
        prep_xf(prep)
        prep_xstats(prep, 0, TI)
        prep_xfinish(prep)
        # batch-0 K/V runs while emb weights stream in
        emit_kv_proj(prep, 0)
        emit_kv_proj(prep, 1)
        emit_kv_attn(prep)

        # ---- emb / FiLM phase (all 4 batches at once) ----
        with tc.tile_pool(name="wemb", bufs=2) as wep, \
             tc.tile_pool(name="etmp", bufs=1) as ep, \
             tc.tile_pool(name="pse", bufs=2, space=bass.MemorySpace.PSUM) as pse:
            if emb_bias:
                embb_r = ep.tile([1, D2], BF16)
                nc.sync.dma_start(embb_r[:], d_embb[None, :])
            fg_r = ep.tile([1, D], BF16)
            nc.sync.dma_start(fg_r[:], d_fg[None, :])
            fb_r = ep.tile([1, D], BF16)
            nc.sync.dma_start(fb_r[:], d_fb[None, :])
            emb_sb = ep.tile([BPC, TE], BF16)
            nc.sync.dma_start(emb_sb[:], d_emb[:])
            # silu(emb) = (emb/2)*(1+tanh(emb/2)) -- stays on the exp table set
            th = ep.tile([BPC, TE], BF16)
            nc.scalar.activation(th[:], emb_sb[:], AF.Tanh, scale=0.5)
            nc.vector.tensor_scalar(th[:], th[:], 0.5, 0.5,
                                    op0=ALU.mult, op1=ALU.add)
            semb = ep.tile([BPC, TE], BF16)
            nc.vector.tensor_mul(semb[:], th[:], emb_sb[:])
            embT = ep.tile([128, KTE, BPC], BF16)
            for c in range(KTE):
                pst = pse.tile([128, BPC], BF16, tag="pst")
                nc.tensor.transpose(pst[:], semb[:, c * 128:(c + 1) * 128],
                                    ident[0:BPC, 0:BPC])
                nc.vector.tensor_copy(embT[:, c, :], pst[:])
            e_sb = ep.tile([BPC, D2], BF16)
            for ch in range(D2 // 512):
                # wemb streamed in 512-col chunks (2MB each) -- the full
                # [TE, 2D] weight would not fit next to the batch pools
                wemb_c = wep.tile([128, KTE, 512], BF16, tag="wembc",
                                  name="wembc")
                nc.sync.dma_start(
                    wemb_c[:],
                    d_wemb[:, ch * 512:(ch + 1) * 512].rearrange(
                        "(i p) m -> p i m", p=128))
                pe = pse.tile([BPC, 512], F32, tag="pe")
                for kt in range(KTE):
                    nc.tensor.matmul(pe[:], embT[:, kt, :],
                                     wemb_c[:, kt, :],
                                     start=(kt == 0),
                                     stop=(kt == KTE - 1 and not emb_bias))
                if emb_bias:
                    nc.tensor.matmul(pe[:], ones_r[0:1, 0:BPC],
                                     embb_r[0:1, ch * 512:(ch + 1) * 512],
                                     start=False, stop=True)
                nc.vector.tensor_copy(e_sb[:, ch * 512:(ch + 1) * 512], pe[:])
            # halved FiLM rows: A' = (fg/2)*(1+scale), B' = (fb/2)*(1+scale)
            #                   + shift/2   (fg/fb arrive pre-halved)
            fg4 = ep.tile([BPC, D], BF16)
            nc.gpsimd.partition_broadcast(fg4[:], fg_r[:], channels=BPC)
            fb4 = ep.tile([BPC, D], BF16)
            nc.gpsimd.partition_broadcast(fb4[:], fb_r[:], channels=BPC)
            tall = ep.tile([BPC, D], BF16)
            nc.vector.tensor_scalar_add(tall[:], e_sb[:, 0:D], 1.0)
            nc.vector.tensor_mul(a_bf[:], tall[:], fg4[:])
            btmp = ep.tile([BPC, D], BF16)
            nc.vector.tensor_mul(btmp[:], tall[:], fb4[:])
            nc.vector.scalar_tensor_tensor(b_bf[:], e_sb[:, D:D2], 0.5,
                                           btmp[:], op0=ALU.mult, op1=ALU.add)

        # ---- remaining weights (after wemb so the FiLM rows are ready
        # before batch 0's first tiles need them) ----
        wq_sb = wpool.tile([128, KD, D], F8)
        nc.sync.dma_start(wq_sb[:], d_wq[:].rearrange("(i p) m -> p i m", p=128))
        wo_sb = wpool.tile([128, KD, D], BF16)
        nc.sync.dma_start(wo_sb[:], d_wo[:].rearrange("(i p) m -> p i m", p=128))

        # ---- t-loop pools (allocated after emb scratch is released) ----
        xnp = ctx.enter_context(tc.tile_pool(name="xn", bufs=2))
        xntp = ctx.enter_context(tc.tile_pool(name="xnT", bufs=3))
        qp = ctx.enter_context(tc.tile_pool(name="q", bufs=2))
        qtp = ctx.enter_context(tc.tile_pool(name="qT", bufs=3))
        hp = ctx.enter_context(tc.tile_pool(name="h", bufs=2))
        htp = ctx.enter_context(tc.tile_pool(name="hT", bufs=3))
        outp = ctx.enter_context(tc.tile_pool(name="o", bufs=2))
        psy = ctx.enter_context(
            tc.tile_pool(name="psy", bufs=2, space=bass.MemorySpace.PSUM))
        pso = ctx.enter_context(
            tc.tile_pool(name="pso", bufs=2, space=bass.MemorySpace.PSUM))

        prep_ab(prep, 0)

        for b in range(BPC):
            x_nat = prep["x_nat"]
            a_bc, b_bc = prep["a_bc"], prep["b_bc"]
            attn_s = prep["attn_s"]
            nxt = None

            # ---------- software-pipelined t-tile loop ----------
            qT_tiles = {}
            hT_tiles = {}
            b1_tiles = {}
            xnT_tiles = prep.pop("xnT_tiles", None)
            if xnT_tiles is None:
                xnT_tiles = {0: emit_xnT(prep, 0)}

            for it in range(TI + 3):
                if b + 1 < BPC and it == 0:
                    nxt = prep_loads(b + 1)
                    prep_ab(nxt, b + 1)

                # ---- prep xn/xnT for tile it+1 ----
                if it + 1 < TI:
                    xnT_tiles[it + 1] = emit_xnT(prep, it + 1)

                # ---- stage B (PE): softmax sums + y-matmul for tile it-1
                #      (first PE group of the iteration: inputs are a full
                #       iteration old, so the PE never waits here) ----
                tj = it - 1
                if 0 <= tj < TI:
                    qT_t = qT_tiles.pop(tj)
                    psS = psa.tile([128, H], F32, tag="psS", name="psS")
                    pys = [psy.tile([128, 512], F32, tag="py", name=f"py{ch}")
                           for ch in range(NCH)]
                    for j in range(KD):
                        nc.tensor.matmul(
                            pys[j // 4][:, (j % 4) * 128:(j % 4) * 128 + 128],
                            qT_t[:, j, :], attn_s[:, j, :],
                            start=True, stop=True)
                        nc.tensor.matmul(psS[:, 2 * j:2 * j + 2],
                                         qT_t[:, j, :], ones2[:],
                                         start=True, stop=True)

                # ---- stage A: Q-proj / exp / qT for tile it ----
                # (softmax normalization is deferred past the y-matmul)
                if it < TI:
                    xnT_t = xnT_tiles.pop(it)
                    exp_q = qp.tile([128, D], BF16, tag="expq", name="expq")
                    for ch in range(NCH):
                        cs = slice(ch * 512, (ch + 1) * 512)
                        pq = psq.tile([128, 512], F32, tag="ps", name="pq")
                        for u in range(KD // 2):
                            nc.tensor.matmul(
                                pq[:], xnT_t[:, 2 * u:2 * u + 2, :],
                                wq_sb[:, 2 * u:2 * u + 2, cs],
                                start=(u == 0),
                                stop=(u == KD // 2 - 1 and not qkv_bias),
                                perf_mode=mybir.MatmulPerfMode.DoubleRow)
                        if qkv_bias:
                            nc.tensor.matmul(pq[:], ones_r[:], bqe_r[0:1, cs],
                                             start=False, stop=True)
                        # wq is pre-scaled by 16 for fp8 range; undo at exp
                        nc.scalar.activation(exp_q[:, cs], pq[:], AF.Exp,
                                             scale=1.0 / 16.0)
                    qT_tiles[it] = qtp.tile([128, KD, 128], BF16, tag="qT",
                                            name="qTt")
                    nc.sync.dma_start_transpose(qT_tiles[it][:], exp_q[:])

                # ---- stage B1: softmax-div + LN stats for tile it-1 ----
                if 0 <= tj < TI:
                    r_q = colp.tile([128, H], F32, tag="rq", name="rq")
                    nc.vector.reciprocal(r_q[:], psS[:])
                    # y / S (grouped broadcast over each head's 64 channels)
                    ynorm = hp.tile([128, D], BF16, tag="ynorm", name="ynorm")
                    for ch in range(NCH):
                        cs = slice(ch * 512, (ch + 1) * 512)
                        sl = r_q[:, ch * 8:(ch + 1) * 8]
                        rq_bc = bass.AP(tensor=sl.tensor, offset=sl.offset,
                                        ap=[[sl.ap[0][0], 128], [1, 8],
                                            [0, C]])
                        nc.vector.tensor_mul(
                            ynorm[:, cs].rearrange("p (h c) -> p h c", c=C),
                            pys[ch][:].rearrange("p (h c) -> p h c", c=C),
                            rq_bc)
                    sty = colp.tile([128, 2, 6], F32, tag="bnsty", name="bnsty")
                    nc.vector.bn_stats(sty[:, 0, :], ynorm[:, 0:512])
                    nc.vector.bn_stats(sty[:, 1, :], ynorm[:, 512:1024])
                    mvy = colp.tile([128, 2], F32, tag="mvy", name="mvy")
                    nc.vector.bn_aggr(mvy[:], sty[:])
                    # rsqrt + -mu*rstd on the otherwise-idle GpSimd engine,
                    # a full pipeline stage before their use in B2
                    rstdy = colp.tile([128, 1], F32, tag="rstdy", name="rstdy")
                    emit_rsqrt(rstdy[:], mvy[:, 1:2], 1, "y", iters=1)
                    nmry = colp.tile([128, 1], F32, tag="nmry", name="nmry")
                    nc.vector.scalar_tensor_tensor(nmry[:], mvy[:, 0:1], -1.0,
                                                   rstdy[:], op0=ALU.mult,
                                                   op1=ALU.mult)
                    b1_tiles[tj] = (ynorm, rstdy, nmry)

                # ---- stage B2: LN/FiLM/silu/hT for tile it-2 ----
                tm = it - 2
                if 0 <= tm < TI:
                    ynorm, rstdy, nmry = b1_tiles.pop(tm)
                    silu_h = hp.tile([128, D], BF16, tag="siluh", name="siluh")
                    for ch in range(NCH):
                        cs = slice(ch * 512, (ch + 1) * 512)
                        stdt = hp.tile([128, 512], BF16, tag="stdt",
                                       name="stdt")
                        nc.vector.tensor_scalar(stdt[:], ynorm[:, cs],
                                                rstdy[:], nmry[:],
                                                op0=ALU.mult, op1=ALU.add)
                        film = hp.tile([128, 512], BF16, tag="film",
                                       name="film")
                        nc.vector.tensor_mul(film[:], stdt[:], a_bc[:, cs])
                        nc.vector.tensor_add(film[:], film[:], b_bc[:, cs])
                        # film is h/2; silu(h) = film*(1+tanh(film))
                        tht = hp.tile([128, 512], BF16, tag="tht", name="tht")
                        nc.scalar.activation(tht[:], film[:], AF.Tanh)
                        nc.vector.scalar_tensor_tensor(
                            silu_h[:, cs], tht[:], 1.0, film[:],
                            op0=ALU.add, op1=ALU.mult)
                    hT_tiles[tm] = htp.tile([128, KD, 128], BF16, tag="hT",
                                            name="hTt")
                    nc.sync.dma_start_transpose(hT_tiles[tm][:], silu_h[:])

                # ---- next-batch prep at the END of the body: the DVE/ACT
                #      FIFOs serve this batch's pipeline first ----
                if b + 1 < BPC:
                    if it == 1:
                        prep_xf_stats(nxt)
                    elif it == 2:
                        prep_xf(nxt)
                    elif 3 <= it <= 6:
                        prep_xstats(nxt, 2 * (it - 3), 2 * (it - 2))
                    elif it == 7:
                        prep_xfinish(nxt)
                    elif it == 8:
                        # next batch's K/V fills the pipeline-drain iters
                        emit_kv_proj(nxt, 0)
                        nxt["xnT_tiles"] = {0: emit_xnT(nxt, 0)}
                    elif it == 9:
                        emit_kv_proj(nxt, 1)
                        emit_kv_attn(nxt)

                # ---- stage C: out-proj + residual + store for tile it-3 ----
                tk = it - 3
                if tk >= 0:
                    hT_t = hT_tiles.pop(tk)
                    o_sb = outp.tile([128, D], BF16, tag="osb", name="osb")
                    for ch in range(NCH):
                        cs = slice(ch * 512, (ch + 1) * 512)
                        po = pso.tile([128, 512], F32, tag="po", name="po")
                        for j in range(KD):
                            nc.tensor.matmul(
                                po[:], hT_t[:, j, :], wo_sb[:, j, cs],
                                start=(j == 0), stop=False)
                        if out_bias:
                            nc.tensor.matmul(po[:], ones_r[:],
                                             outb_r[0:1, cs],
                                             start=False, stop=False)
                        # residual: out += I^T @ x adds x via the PE
                        nc.tensor.matmul(po[:], ident[:], x_nat[:, tk, cs],
                                         start=False, stop=True)
                        nc.scalar.copy(o_sb[:, cs], po[:])
                    nc.sync.dma_start(
                        d_out[b, tk * 128:(tk + 1) * 128, :], o_sb[:])

            if nxt is not None:
                prep = nxt

    nc.compile()
    return nc


def _get_program(qkv_bias, out_bias, emb_bias):
    key = (qkv_bias, out_bias, emb_bias)
    if key not in _PROGRAMS:
        _PROGRAMS[key] = _build_program(qkv_bias, out_bias, emb_bias)
    return _PROGRAMS[key]


def _prep_inputs(inputs):
    f = lambda k: np.asarray(inputs[k], np.float32)
    x, xf, emb = f("x"), f("xf"), f("emb")
    norm_g, norm_b = f("norm_g"), f("norm_b")
    tnorm_g, tnorm_b = f("tnorm_g"), f("tnorm_b")
    Wq, bq, Wk, bk, Wv, bv = f("Wq"), f("bq"), f("Wk"), f("bk"), f("Wv"), f("bv")
    emb_W, emb_b = f("emb_W"), f("emb_b")
    fg, fb = f("fnorm_g"), f("fnorm_b")
    out_W, out_b = f("out_W"), f("out_b")

    wq_e = norm_g[:, None] * Wq
    wk_e = tnorm_g[:, None] * Wk
    wv_e = tnorm_g[:, None] * Wv
    bqe = bq + norm_b @ Wq
    bke = bk + tnorm_b @ Wk
    bve = bv + tnorm_b @ Wv
    qkv_bias = bool(np.any(bqe) or np.any(bke) or np.any(bve))
    out_bias = bool(np.any(out_b))
    emb_bias = bool(np.any(emb_b))
    shared = {
        "wq": (wq_e * 16.0).astype(NF8), "wk": wk_e.astype(NBF), "wv": wv_e.astype(NBF),
        "wo": out_W.astype(NBF), "wemb": emb_W.astype(NBF),
        "bqe": (bqe * 16.0).astype(NBF), "bke": bke.astype(NBF), "bve": bve.astype(NBF),
        "outb": out_b.astype(NBF), "embb": emb_b.astype(NBF),
        "fg": (fg * 0.5).astype(NBF), "fb": (fb * 0.5).astype(NBF),
    }
    xbf = x.astype(NBF)
    xfbf = xf.astype(NBF)
    in_maps = []
    for i in range(NCORES):
        s = slice(i * BPC, (i + 1) * BPC)
        m = dict(shared)
        m["xbf"] = xbf[s]
        m["xfbf"] = xfbf[s]
        m["emb"] = emb[s].astype(NBF)
        in_maps.append(m)
    return in_maps, qkv_bias, out_bias, emb_bias


def run(inputs, trace=False):
    in_maps, qkv_bias, out_bias, emb_bias = _prep_inputs(inputs)
    nc = _get_program(qkv_bias, out_bias, emb_bias)
    res = run_bass_kernel_spmd(nc, in_maps, core_ids=list(range(NCORES)),
                               trace=trace)
    out = np.concatenate(
        [res.results[i]["out"].astype(np.float32) for i in range(NCORES)],
        axis=0)
    return out, res


def kernel(**inputs):
    out, _ = run(inputs, trace=False)
    return out
